# revision 17
# baseline (speedup 1.0000x reference)
"""MultiHeadCrossAttention kernel for 8 trn2 NeuronCores.

Reference computation (fp32, per batch b):
    q = Q[b] @ W_q.T ; k = K[b] @ W_k.T ; v = V[b] @ W_v.T      (heads on columns)
    per head h: S = (q_h @ k_h.T) / 8 ; E = exp(S); A = E / E.sum(-1)
    out[b] = concat_h(A @ v_h) @ W_o.T ; rows with mask==0 zeroed

Sharding: 8 cores = (batch b in {0,1}) x (head-group hg in {0..3}, 4 heads each).
Each core computes a partial output  out_part[b] = concat(heads hg) @ W_o[:, cols].T
and the host sums the 4 partials per batch (bf16 partials, fp32 host sum).

Design: the kernel is ScalarE-bound (exp over 4 heads x 2048 x 2048 = 16.8M
elements at 1 elem/lane/cycle @ 1.2 GHz ~= 147 us).  Everything else is
scheduled to hide under the exp stream:
  - Minimal prologue: chunked input DMAs (512-seq-position chunks, host
    pre-packed so every DMA is contiguous 8KB-per-partition), k/q projections
    for the first block only; first exp by ~7 us.
  - All remaining projections, W_o matmuls, and normalizations are emitted as
    deadline-scheduled background items interleaved into the attention j-loop
    (the PE has ~40% slack under the exp stream).
  - Deep e-tile buffering (12 bufs) lets ScalarE run ahead of the PV matmuls
    so transient PE oversubscription doesn't stall the exp stream.
  - Scores run as row-tiled concurrent matmul pairs (two heads, K=64 each, PE
    rows 0:64 / 64:128).  PV uses the augmented-V trick (stationary column 0 =
    ones -> PSUM row 0 accumulates the softmax denominator).
  - Reciprocal broadcast across partitions via a K=1 PE outer-product (no DRAM
    round trip).  ScalarE does nothing but the 128 exp activations.
"""

import numpy as np
import ml_dtypes

import concourse.bass as bass
import concourse.bacc as bacc
import concourse.mybir as mybir
import concourse.tile as tile
from contextlib import ExitStack

F32 = mybir.dt.float32
BF16 = mybir.dt.bfloat16
AF = mybir.ActivationFunctionType

B = 2
SEQ = 2048          # Sq == Sk
D = 1024            # model dim
DL = 256            # local head dims per core (4 heads x 64)
HL = 4              # local heads
DH = 64             # head dim
NCORES = 8

_PROGRAM = None


def build_program():
    nc = bacc.Bacc("TRN2", target_bir_lowering=False)

    # x inputs host-packed as [p, c*8+a, s]: element (model a*128+p, seq c*512+s)
    xq = nc.declare_dram_parameter("xq", [128, 32, 512], BF16, isOutput=False)
    xk = nc.declare_dram_parameter("xk", [128, 32, 512], BF16, isOutput=False)
    xv = nc.declare_dram_parameter("xv", [128, 32, 512], BF16, isOutput=False)
    # weights host-packed [p, a, d]: element (model a*128+p, d)
    wq = nc.declare_dram_parameter("wq", [128, 8, DL], BF16, isOutput=False)
    wk = nc.declare_dram_parameter("wk", [128, 8, DL], BF16, isOutput=False)
    wv = nc.declare_dram_parameter("wv", [128, 8, DL], BF16, isOutput=False)
    wo = nc.declare_dram_parameter("wo", [128, 2, D], BF16, isOutput=False)
    maskf = nc.declare_dram_parameter("maskf", [128, SEQ // 128], F32, isOutput=False)
    out_part = nc.declare_dram_parameter("out_part", [SEQ, D], BF16, isOutput=True)

    with tile.TileContext(nc) as tc, ExitStack() as ctx:
        const = ctx.enter_context(tc.tile_pool(name="const", bufs=1))
        proj = ctx.enter_context(tc.tile_pool(name="proj", bufs=1))
        xkp = ctx.enter_context(tc.tile_pool(name="xkp", bufs=4))
        xqp = ctx.enter_context(tc.tile_pool(name="xqp", bufs=4))
        xvp = ctx.enter_context(tc.tile_pool(name="xvp", bufs=4))
        epool = ctx.enter_context(tc.tile_pool(name="epool", bufs=8))
        opool = ctx.enter_context(tc.tile_pool(name="opool", bufs=4))
        ospool = ctx.enter_context(tc.tile_pool(name="ospool", bufs=2))
        rpool = ctx.enter_context(tc.tile_pool(name="rpool", bufs=2))
        pp = ctx.enter_context(tc.tile_pool(name="pp", bufs=2, space="PSUM"))
        stp = ctx.enter_context(tc.tile_pool(name="stp", bufs=2, space="PSUM"))
        accp = ctx.enter_context(tc.tile_pool(name="accp", bufs=2, space="PSUM"))

        # ---------------- constants ----------------
        wq_sb = const.tile([128, 8, DL], BF16)
        wk_sb = const.tile([128, 8, DL], BF16)
        wv_sb = const.tile([128, 8, DL], BF16)
        wo_sb = const.tile([128, 2, D], BF16)
        mask_sb = const.tile([128, SEQ // 128], F32)
        ones_sb = const.tile([1, 64], F32)
        nc.vector.memset(ones_sb[:], 1.0)
        # critical-path weights first; wo/mask are posted later (bg schedule)
        nc.sync.dma_start(wk_sb[:], wk[:])
        nc.sync.dma_start(wq_sb[:], wq[:])
        nc.sync.dma_start(wv_sb[:], wv[:])

        kTs = (
            proj.tile([128, SEQ], BF16, name="kT0"),
            proj.tile([128, SEQ], BF16, name="kT1"),
        )
        qTs = [
            [proj.tile([128, 512], BF16, name=f"qT{dm}_{qp}") for qp in range(4)]
            for dm in range(2)
        ]
        vaugs = [
            proj.tile([128, HL, 128], BF16, name=f"vaug{j}") for j in range(16)
        ]

        def vmemset(j):
            # col 0 = ones (denominator row); cols 1:64 zero; 64:128 = v
            # (written by vproj).  acc rows 1:63 are never read.
            nc.vector.memset(vaugs[j][:, :, 0:64], 0.0)
            nc.vector.memset(vaugs[j][:, :, 0:1], 1.0)

        # ---------------- projection emitters ----------------
        x_tiles = {}

        def dma_x(which, c):
            src = {"k": xk, "q": xq, "v": xv}[which]
            pool = {"k": xkp, "q": xqp, "v": xvp}[which]
            t = pool.tile([128, 8, 512], BF16, tag=f"x{which}", name=f"x{which}{c}")
            nc.sync.dma_start(t[:], src[:, c * 8 : (c + 1) * 8, :])
            x_tiles[(which, c)] = t

        def kproj(dm, c):
            x_t = x_tiles[("k", c)]
            ps = pp.tile([128, 512], F32, tag="pp")
            for ki in range(8):
                nc.tensor.matmul(
                    ps[:],
                    lhsT=wk_sb[:, ki, dm * 128 : (dm + 1) * 128],
                    rhs=x_t[:, ki, :],
                    start=(ki == 0),
                    stop=(ki == 7),
                )
            nc.vector.tensor_copy(kTs[dm][:, c * 512 : (c + 1) * 512], ps[:])

        def qproj(dm, qp):
            x_t = x_tiles[("q", qp)]
            ps = pp.tile([128, 512], F32, tag="pp")
            for ki in range(8):
                nc.tensor.matmul(
                    ps[:],
                    lhsT=wq_sb[:, ki, dm * 128 : (dm + 1) * 128],
                    rhs=x_t[:, ki, :],
                    start=(ki == 0),
                    stop=(ki == 7),
                )
            nc.vector.tensor_copy(qTs[dm][qp][:], ps[:])

        def vproj(j, hg):
            # v projection for 128 seq positions (chunk j), head pair hg
            c, km = j // 4, j % 4
            x_t = x_tiles[("v", c)]
            ps = pp.tile([128, 512], F32, tag="pp")
            for ki in range(8):
                nc.tensor.matmul(
                    ps[:, 0:128],
                    lhsT=x_t[:, ki, km * 128 : (km + 1) * 128],
                    rhs=wv_sb[:, ki, hg * 128 : (hg + 1) * 128],
                    start=(ki == 0),
                    stop=(ki == 7),
                )
            nc.vector.tensor_copy(
                vaugs[j][:, 2 * hg : 2 * hg + 2, 64 : 64 + DH],
                ps[:, 0:128].rearrange("p (h d) -> p h d", h=2),
            )

        # ---------------- output-side emitters ----------------
        outTs = [opool.tile([128, 2, 512], BF16, name=f"outT{qp}") for qp in range(4)]

        def normalize(qp, hp, hi, acc_ps):
            acc_sb = rpool.tile([128, 512], F32, tag="accsb")
            nc.vector.tensor_copy(acc_sb[:], acc_ps[:])
            r_sb = rpool.tile([1, 512], F32, tag="r")
            nc.vector.reciprocal_approx_fast(out=r_sb[:], in_=acc_sb[0:1, :])
            rb_ps = pp.tile([64, 512], F32, tag="pp", name=f"rb{qp}_{hp}_{hi}")
            nc.tensor.matmul(
                rb_ps[:], lhsT=ones_sb[:], rhs=r_sb[:], start=True, stop=True
            )
            nc.vector.tensor_mul(
                outTs[qp][hi * 64 : (hi + 1) * 64, hp, :],
                acc_sb[64 : 64 + DH, :],
                rb_ps[:],
            )

        def wo_group(qp, mq):
            qg = qp * 4 + mq
            o_sb = ospool.tile([128, D], BF16, tag="o", name=f"wo_o{qp}_{mq}")
            for oc in range(2):
                ps = pp.tile([128, 512], F32, tag="pp", name=f"wops{qp}_{mq}_{oc}")
                for kc in range(2):
                    nc.tensor.matmul(
                        ps[:],
                        lhsT=outTs[qp][:, kc, mq * 128 : (mq + 1) * 128],
                        rhs=wo_sb[:, kc, oc * 512 : (oc + 1) * 512],
                        start=(kc == 0),
                        stop=(kc == 1),
                    )
                nc.vector.tensor_scalar_mul(
                    o_sb[:, oc * 512 : (oc + 1) * 512], ps[:], mask_sb[:, qg : qg + 1]
                )
            nc.sync.dma_start(out_part[qg * 128 : (qg + 1) * 128, :], o_sb[:])

        # ---------------- static background schedule ----------------
        # bg[(block, step)] emitted between the exp and the PV matmuls of that
        # j-step, so background work never delays the exp critical path but
        # fills the PE while ScalarE runs.
        bg = {}

        def sched(block, step, fn):
            bg.setdefault((block, step), []).append(fn)

        # staggered input DMA posts (posting everything up front makes the HW
        # queues split bandwidth evenly and starves the critical path)
        sched(0, 0, lambda: dma_x("k", 2))
        sched(0, 2, lambda: dma_x("v", 1))
        sched(0, 4, lambda: dma_x("k", 3))
        sched(0, 6, lambda: dma_x("q", 1))
        sched(0, 4, lambda: dma_x("v", 2))
        sched(0, 12, lambda: nc.sync.dma_start(wo_sb[:], wo[:]))
        sched(0, 12, lambda: nc.sync.dma_start(mask_sb[:], maskf[:]))
        sched(0, 8, lambda: dma_x("v", 3))
        sched(1, 8, lambda: dma_x("q", 2))
        sched(2, 8, lambda: dma_x("q", 3))
        # block 0: v (head pair 0, chunks 0-1) + remaining kT(dm0) + qT(dm0,qp1)
        for s, j in zip(range(0, 8), range(2, 10)):
            sched(0, s, (lambda jj: lambda: vproj(jj, 0))(j))
        sched(0, 9, lambda: vproj(10, 0))
        sched(0, 10, lambda: vproj(11, 0))
        sched(0, 12, lambda: vproj(12, 0))
        sched(0, 12, lambda: vproj(13, 0))
        sched(0, 13, lambda: vproj(14, 0))
        sched(0, 14, lambda: vproj(15, 0))
        sched(0, 3, lambda: kproj(0, 1))
        sched(0, 7, lambda: kproj(0, 2))
        sched(0, 11, lambda: kproj(0, 3))
        sched(0, 15, lambda: qproj(0, 1))
        for j in range(4, 16):
            sched(0, j - 2, (lambda jj: lambda: vmemset(jj))(j))
        # block 1: kT(dm1) while the xk chunks are still resident
        sched(1, 2, lambda: kproj(1, 0))
        sched(1, 5, lambda: kproj(1, 1))
        sched(1, 8, lambda: kproj(1, 2))
        sched(1, 11, lambda: kproj(1, 3))
        sched(1, 14, lambda: qproj(0, 2))
        # blocks 2-3: v (head pair 1), later q projections
        for i, j in enumerate(range(0, 8)):
            sched(2, 1 + i, (lambda jj: lambda: vproj(jj, 1))(j))
        for i, j in enumerate(range(8, 16)):
            sched(3, 1 + i, (lambda jj: lambda: vproj(jj, 1))(j))
        sched(2, 14, lambda: qproj(0, 3))
        sched(3, 12, lambda: qproj(1, 0))
        sched(4, 14, lambda: qproj(1, 1))
        sched(5, 14, lambda: qproj(1, 2))
        sched(6, 14, lambda: qproj(1, 3))
        # W_o for qp emitted during block 4+qp+1 (scheduled dynamically below
        # because they must be emitted after normalize(qp, 1, *))
        for qp in range(3):
            for mq in range(4):
                sched(5 + qp, 1 + 2 * mq, (lambda q, m: lambda: wo_group(q, m))(qp, mq))

        # ---------------- prologue ----------------
        vmemset(0)
        vmemset(1)
        dma_x("k", 0)
        dma_x("q", 0)
        dma_x("v", 0)
        dma_x("k", 1)
        kproj(0, 0)
        qproj(0, 0)
        vmemset(2)
        vmemset(3)
        vproj(0, 0)
        vproj(1, 0)

        # ---------------- main loop ----------------
        for block in range(8):
            hp, qp = block // 4, block % 4
            acc = [
                accp.tile([128, 512], F32, tag="acc", name=f"acc{hp}_{qp}_{i}")
                for i in range(2)
            ]
            for j in range(16):
                st = stp.tile([128, 1024], F32, tag="st")
                for hi in range(2):
                    r0 = hi * 64
                    nc.tensor.matmul(
                        st[:, hi * 512 : (hi + 1) * 512],
                        lhsT=kTs[hp][r0 : r0 + 64, j * 128 : (j + 1) * 128],
                        rhs=qTs[hp][qp][r0 : r0 + 64, :],
                        start=True,
                        stop=True,
                    )
                e_t = epool.tile([128, 1024], BF16, tag="e")
                nc.scalar.activation(out=e_t[:], in_=st[:], func=AF.Exp, scale=0.125)
                for fn in bg.get((block, j), ()):
                    fn()
                for hi in range(2):
                    h = 2 * hp + hi
                    nc.tensor.matmul(
                        acc[hi][:],
                        lhsT=vaugs[j][:, h, :],
                        rhs=e_t[:, hi * 512 : (hi + 1) * 512],
                        start=(j == 0),
                        stop=(j == 15),
                    )
            for hi in range(2):
                normalize(qp, hp, hi, acc[hi])
        # tail: W_o of the last qp
        for mq in range(4):
            wo_group(3, mq)

    nc.compile()
    return nc


def _get_program():
    global _PROGRAM
    if _PROGRAM is None:
        _PROGRAM = build_program()
    return _PROGRAM


def _pack_x(x):
    # [2048 seq, 1024 model] -> [p, c*8+a, s]: element (model a*128+p, seq c*512+s)
    xt = np.ascontiguousarray(x.T).reshape(8, 128, 4, 512)
    return np.ascontiguousarray(xt.transpose(1, 2, 0, 3).reshape(128, 32, 512))


def _pack_w(wt, a):
    # [a*128 contraction, d] -> [p, a, d]: element (a*128+p, d)
    return np.ascontiguousarray(wt.reshape(a, 128, wt.shape[1]).transpose(1, 0, 2))


def make_in_maps(Q, K, V, mask, W_q, W_k, W_v, W_o):
    bf = ml_dtypes.bfloat16
    Q, K, V = (np.asarray(a, np.float32) for a in (Q, K, V))
    W_q, W_k, W_v, W_o = (np.asarray(a, np.float32) for a in (W_q, W_k, W_v, W_o))
    mask = np.asarray(mask)
    in_maps = []
    for core in range(NCORES):
        b, hg = core // 4, core % 4
        c0 = hg * DL
        in_maps.append(
            {
                "xq": _pack_x(Q[b]).astype(bf),
                "xk": _pack_x(K[b]).astype(bf),
                "xv": _pack_x(V[b]).astype(bf),
                "wq": _pack_w(W_q[c0 : c0 + DL, :].T, 8).astype(bf),
                "wk": _pack_w(W_k[c0 : c0 + DL, :].T, 8).astype(bf),
                "wv": _pack_w(W_v[c0 : c0 + DL, :].T, 8).astype(bf),
                "wo": _pack_w(W_o[:, c0 : c0 + DL].T, 2).astype(bf),
                "maskf": np.ascontiguousarray(
                    mask[b].reshape(SEQ // 128, 128).T
                ).astype(np.float32),
            }
        )
    return in_maps


def gather(results):
    out = np.zeros((B, SEQ, D), np.float32)
    for core in range(NCORES):
        out[core // 4] += results[core]["out_part"].astype(np.float32)
    return out


def kernel(Q, K, V, mask, W_q, W_k, W_v, W_o):
    from concourse.bass_utils import run_bass_kernel_spmd

    nc = _get_program()
    in_maps = make_in_maps(Q, K, V, mask, W_q, W_k, W_v, W_o)
    res = run_bass_kernel_spmd(nc, in_maps, list(range(NCORES))).results
    return gather(res)
